# revision 5
# baseline (speedup 1.0000x reference)
"""CRF Viterbi decode kernel for Trainium2 (8 NeuronCores, data-parallel over batch).

Per core (128 sequences, batch on partitions):
  Phase A: DMA X slabs (front/back interleaved) -> PE transpose -> PE matmul with W
           -> emissions e[b, s, l] (ACT copies PSUM->SBUF).
  Fused scans: forward delta-scan and backward beta-scan advance in lockstep
           pairs.  Each direction's "broadcast-add + segmented max" is ONE
           custom DVE instruction (SEG_MAXSCAN_ANT: scan(MAX, Src0+Src1) with a
           hand-built SUB_DIM_DONE page-reset state), so a pair costs
           2x ~900ns + one [128,52] add instead of ~3.3us of stock ops.
           GpSimd persists beta (first half) and computes gamma = delta + beta
           in-place into dead e-slots (both halves) during the scan.
  Tail:    onehot = (gamma >= rowmax(gamma)); DMA out.

Matches the jax reference up to f32 summation-order fuzz in emissions.
"""

import numpy as np

B, S, D, L = 1024, 512, 128, 26
NCORES = 8
BC = B // NCORES  # 128 sequences per core

_BUILD_CACHE = {}


# --------------------------------------------------------------------------
# Custom DVE op: fused broadcast-add + segmented max-scan.
#
# out[p, s, n] = running max over n' <= n of (in0[p, s, n'] + in1[p, s, n'])
# with the running max RESET at each page boundary, so out[p, s, N-1] =
# max_n (in0[p,s,n] + in1[p,s,n]) -- a fused add + segmented max reduce in
# one 1-elem/cycle DVE pass.
#
# The stock dve_spec.lower() only emits a SUB_DIM step state for
# PageIdx-style scans; a plain scan() accumulates globally across pages.
# Here we hand-build the FSM: steady fires SUB_DIM_DONE -> step; the step
# state (one element, the first of the new page) computes
# d = MAX(MaxNeg, expr) = expr, i.e. reset-and-include, then returns to
# steady.  Verified bit-exact on hardware (see session microbench).
# --------------------------------------------------------------------------


def _np_segmax(in0, in1, c0, c1, c2):
    s = in0 + in1
    return np.maximum.accumulate(s, axis=-1)


def _register_segmax():
    from concourse import dve_spec as Dv
    from concourse import dve_ops as DO
    from concourse.dve_spec import Spec, Src0, Src1, scan, AluOp
    from concourse.dve_uop import DveOpSpec

    for op in DO.OPS:
        if op.name == "SEG_MAXSCAN_ANT":
            return op

    def _lower_segmented(spec, ver):
        Dv._validate_body(spec, ver)
        spec2 = Dv._hoist_stream_invariant_ops(spec)
        scans = Dv._collect(spec2.body, Dv.Scan)
        latches = Dv._collect(spec2.body, Dv.Latch)
        assert not latches
        p = Dv._build_placement(spec2, scans, Dv.N_STAGES[ver], Dv.N_LANES[ver])
        states = list(Dv._build_state_machine(spec2, scans, latches, p))
        assert len(states) == 2, states  # [seed, steady]
        consume = states[1].consume

        step_ov = {}
        for sc in scans:
            d = p.node_stage[sc]
            init = Dv._scan_init(sc)
            # steady stage is _Stage(op, CURR_ALU_OUT, sc.expr); replace the
            # accumulator operand with the init leaf: d = op(init, expr)
            step_ov[d] = Dv._Stage(sc.op, init, sc.expr)

        steady_idx, step_idx = 1, 2
        states[steady_idx] = Dv._State(
            placement=p,
            consume=consume,
            trigger=(Dv.Trigger.SRC_TENSOR_DONE, Dv.Trigger.SUB_DIM_DONE,
                     Dv.Trigger.NONE),
            next=(0, step_idx, 0),
        )
        states.append(
            Dv._State(
                placement=p,
                consume=consume,
                overrides=step_ov,
                trigger=(Dv.Trigger.SRC_TENSOR_DONE, Dv.Trigger.SUB_DIM_DONE,
                         Dv.Trigger.COUNT),
                next=(0, step_idx, steady_idx),
                repeat=1,
            )
        )
        out = [Dv._assemble(s) for s in states]
        for u in out:
            u.validate(ver)
        return out

    class SegDveOp(DO.DveOp):
        def compile(self, ver):
            key = (self.name, ver)
            if (r := DO._COMPILE_CACHE.get(key)) is not None:
                return r
            result = DveOpSpec(
                name=self.name,
                opcode=DO.get_dve_sub_opcode(self.name),
                uops=_lower_segmented(self.spec, ver),
                rd1_en=DO.has_src1(self.spec),
            )
            DO._COMPILE_CACHE[key] = result
            return result

    spec = Spec(body=scan(AluOp.MAX, Src0 + Src1), reference=_np_segmax)
    op = SegDveOp("SEG_MAXSCAN_ANT", spec, subdim=True, uops_sha={})
    DO.OPS.append(op)
    DO._SUB_OPCODE_FOR_NAME[op.name] = DO._CUSTOM_DVE_ROW_BASE + len(DO.OPS) - 1
    assert DO._SUB_OPCODE_FOR_NAME[op.name] < 0x20
    return op


def _build(s_len):
    import concourse.bass as bass
    import concourse.bacc as bacc
    import concourse.tile as tile
    import concourse.mybir as mybir

    segmax = _register_segmax()

    alu = mybir.AluOpType
    f32 = mybir.dt.float32
    i32 = mybir.dt.int32

    nc = bacc.Bacc("TRN2", target_bir_lowering=False, debug=False)
    Xh = nc.dram_tensor("X", (BC, s_len, D), f32, kind="ExternalInput")
    Wh = nc.dram_tensor("W", (D, L), f32, kind="ExternalInput")
    Th = nc.dram_tensor("T", (L, L), f32, kind="ExternalInput")
    Oh = nc.dram_tensor("OUT", (BC, s_len, L), f32, kind="ExternalOutput")

    SCHUNK = 8 if s_len % 16 == 0 else s_len   # X staging granularity
    NCH = s_len // SCHUNK
    HALF = s_len // 2
    CH = 64 if s_len % 64 == 0 else s_len      # tail chunk (steps)
    NP = s_len - 1                              # number of fused pairs
    CS = s_len                                  # c-slot base index in d_store

    def ap_of(t, offset_elems, dims):
        a = t[:]
        return bass.AP(tensor=a.tensor, offset=a.offset + offset_elems,
                       ap=[list(a.ap[0])] + dims)

    with tile.TileContext(nc) as tc:
        with (
            tc.tile_pool(name="singles", bufs=1) as singles,
            tc.tile_pool(name="xstage", bufs=2) as xstage_p,
            tc.tile_pool(name="xt", bufs=3) as xt_p,
            tc.tile_pool(name="ps_t", bufs=2, space="PSUM") as ps_t,
            tc.tile_pool(name="ps_e", bufs=2, space="PSUM") as ps_e,
            tc.tile_pool(name="scores", bufs=3) as scores_p,
            tc.tile_pool(name="tail", bufs=2) as tail_p,
        ):
            # ---- storage ----
            e_store = singles.tile([BC, s_len * L], f32)   # emissions; then gammas
            d_store = singles.tile([BC, (s_len + 2) * L], f32)  # deltas + 2 c-slots
            b_store = singles.tile([BC, (s_len - HALF) * L], f32)  # beta, t >= HALF
            w_sb = singles.tile([D, L], f32)
            nc.sync.dma_start(w_sb[:], Wh[:])

            # T_cat[p, 0, j, i] = T[i, j] (fwd: pages i, elems j over T^T)
            # T_cat[p, 1, i, j] = T[i, j] (bwd: pages i, elems j over T)
            # DMA only T (contiguous per partition); T^T via ONE on-chip
            # strided copy.  (A per-column gather DMA here costs ~87K 4-byte
            # descriptors and jams all DMA queues for ~100us.)
            t_ap = Th[:]
            t_cat = singles.tile([BC, 2, L, L], f32)
            nc.sync.dma_start(
                t_cat[:, 1, :, :].rearrange("p a b -> p (a b)"),
                bass.AP(tensor=t_ap.tensor, offset=t_ap.offset,
                        ap=[[0, BC], [1, L * L]]),
            )
            tt_in = ap_of(t_cat, L * L, [[1, L], [L, L]])
            nc.vector.tensor_copy(t_cat[:, 0, :, :], tt_in)

            # identity matrix for PE transpose: ident[p, q] = (p == q)
            idx_i = singles.tile([BC, D], i32)
            nc.gpsimd.iota(idx_i[:], pattern=[[1, D]], base=0, channel_multiplier=0)
            pid_i = singles.tile([BC, 1], i32)
            nc.gpsimd.iota(pid_i[:], pattern=[[0, 1]], base=0, channel_multiplier=1)
            idx_f = singles.tile([BC, D], f32)
            nc.vector.tensor_copy(idx_f[:], idx_i[:])
            pid_f = singles.tile([BC, 1], f32)
            nc.vector.tensor_copy(pid_f[:], pid_i[:])
            ident = singles.tile([BC, D], f32)
            nc.vector.tensor_scalar(
                out=ident[:], in0=idx_f[:], scalar1=pid_f[:], scalar2=None,
                op0=alu.is_equal,
            )

            e3 = e_store.rearrange("p (s l) -> p s l", l=L)
            d3 = d_store.rearrange("p (s l) -> p s l", l=L)
            b3 = b_store.rearrange("p (s l) -> p s l", l=L)

            # ---- Phase A: emissions, front/back interleaved chunk order ----
            order = []
            for c in range(NCH // 2):
                order += [c, NCH - 1 - c]
            if NCH % 2:
                order.append(NCH // 2)
            for cidx in order:
                c0 = cidx * SCHUNK
                xs = xstage_p.tile([BC, SCHUNK, D], f32)
                nc.sync.dma_start(xs[:], Xh[:, c0:c0 + SCHUNK, :])
                for si in range(SCHUNK):
                    s = c0 + si
                    xt_psum = ps_t.tile([D, BC], f32)
                    nc.tensor.transpose(xt_psum[:], xs[:, si, :], ident[:])
                    xt_sb = xt_p.tile([D, BC], f32)
                    nc.scalar.copy(xt_sb[:], xt_psum[:])
                    e_psum = ps_e.tile([BC, L], f32)
                    nc.tensor.matmul(e_psum[:], lhsT=xt_sb[:], rhs=w_sb[:],
                                     start=True, stop=True)
                    nc.scalar.copy(e3[:, s, :], e_psum[:])

            # ---- init: delta_0 = e_0; c-slot(0) = beta_{S-1} + e_{S-1} = e_{S-1}
            nc.vector.tensor_copy(d3[:, 0, :], e3[:, 0, :])
            nc.vector.tensor_copy(d3[:, CS, :], e3[:, s_len - 1, :])

            # ---- fused forward/backward scan pairs ----
            # DVE per pair: 2 custom segmax ops + 1 fused [2L] add.
            # GpSimd: beta persist (first half) + gamma = delta + beta parking
            # into dead e-slots (second half).
            for k in range(NP):
                ft = k + 1          # forward step being produced (delta_ft)
                bt = s_len - 2 - k  # backward step being produced (beta_bt)
                cin = CS + (k % 2)
                cout = CS + ((k + 1) % 2)

                sc = scores_p.tile([BC, 2, L, L], f32, tag="sc")
                in1_f = ap_of(d_store, k * L, [[0, L], [1, L]])
                in1_b = ap_of(d_store, cin * L, [[0, L], [1, L]])
                nc.vector._custom_dve(segmax, out=sc[:, 0], in0=t_cat[:, 0],
                                      in1=in1_f)
                nc.vector._custom_dve(segmax, out=sc[:, 1], in0=t_cat[:, 1],
                                      in1=in1_b)

                # [delta_ft | c_next] = page-end maxes + [e_ft | e_bt]
                ends = ap_of(sc, L - 1, [[L * L, 2], [L, L]])
                out_ap = ap_of(d_store, ft * L, [[(cout - ft) * L, 2], [1, L]])
                e_ap = ap_of(e_store, ft * L, [[(bt - ft) * L, 2], [1, L]])
                nc.vector.tensor_tensor(out_ap, ends, e_ap, op=alu.add)

                mxb = ap_of(sc, L * L + L - 1, [[L, L]])  # bwd page-end maxes
                if bt >= HALF:
                    # persist beta_bt for the forward side's gamma later
                    nc.gpsimd.tensor_copy(b3[:, bt - HALF, :], mxb)
                else:
                    # delta_bt is known: gamma_bt = delta_bt + beta_bt
                    nc.gpsimd.tensor_tensor(e3[:, bt, :], d3[:, bt, :], mxb,
                                            op=alu.add)
                if k >= HALF - 1 and ft <= s_len - 2:
                    # gamma_ft = delta_ft + beta_ft (beta from b_store)
                    nc.gpsimd.tensor_tensor(e3[:, ft, :], d3[:, ft, :],
                                            b3[:, ft - HALF, :], op=alu.add)

            # gamma_{S-1} = delta_{S-1} (beta = 0)
            nc.gpsimd.tensor_copy(e3[:, s_len - 1, :], d3[:, s_len - 1, :])

            # ---- Tail: onehot = (gamma >= rowmax(gamma)); DMA out ----
            # Vector does the segmented rowmax (only it can); the IS_GE of
            # early chunks goes to GpSimd so the two engines overlap.
            nchunks = s_len // CH
            for ci, c0 in enumerate(range(0, s_len, CH)):
                gsrc = e3[:, c0:c0 + CH, :]
                gm = tail_p.tile([BC, CH], f32, tag="gm")
                nc.vector.reduce_max(gm[:], gsrc, axis=mybir.AxisListType.X)
                oh = tail_p.tile([BC, CH, L], f32, tag="oh")
                gm_bc = (
                    gm[:]
                    .rearrange("p (t o) -> p t o", o=1)
                    .broadcast_to((BC, CH, L))
                )
                nc.vector.tensor_tensor(oh[:], gsrc, gm_bc, op=alu.is_ge)
                nc.sync.dma_start(Oh[:, c0:c0 + CH, :], oh[:])

    nc.compile()
    return nc


def _get(s_len):
    if s_len not in _BUILD_CACHE:
        _BUILD_CACHE[s_len] = _build(s_len)
    return _BUILD_CACHE[s_len]


LAST_RESULT = None


def kernel(X, W, T):
    global LAST_RESULT
    from concourse.bass_utils import run_bass_kernel_spmd

    X = np.ascontiguousarray(X, dtype=np.float32)
    W = np.ascontiguousarray(W, dtype=np.float32)
    T = np.ascontiguousarray(T, dtype=np.float32)
    s_len = X.shape[1]
    nc = _get(s_len)
    in_maps = [
        {"X": X[c * BC:(c + 1) * BC], "W": W, "T": T} for c in range(NCORES)
    ]
    res = run_bass_kernel_spmd(nc, in_maps, core_ids=list(range(NCORES)))
    LAST_RESULT = res
    return np.concatenate([r["OUT"] for r in res.results], axis=0)


# revision 6
# speedup vs baseline: 1.1109x; 1.1109x over previous
"""CRF Viterbi decode kernel for Trainium2 (8 NeuronCores, data-parallel over batch).

Per core (128 sequences, batch on partitions):
  Phase A: DMA X slabs (front/back interleaved) -> PE transpose -> PE matmul with W
           -> emissions e[b, s, l] (ACT copies PSUM->SBUF).
  Fused scans: forward delta-scan and backward beta-scan advance in lockstep
           pairs.  Each direction's "broadcast-add + segmented max" is ONE
           custom DVE instruction (SEG_MAXSCAN_ANT: scan(MAX, Src0+Src1) with a
           hand-built SUB_DIM_DONE page-reset state), so a pair costs
           2x ~900ns + one [128,52] add instead of ~3.3us of stock ops.
           GpSimd persists beta (first half) and computes gamma = delta + beta
           in-place into dead e-slots (both halves) during the scan.
  Tail:    onehot = (gamma >= rowmax(gamma)); DMA out.

Matches the jax reference up to f32 summation-order fuzz in emissions.
"""

import numpy as np

B, S, D, L = 1024, 512, 128, 26
NCORES = 8
BC = B // NCORES  # 128 sequences per core

_BUILD_CACHE = {}


# --------------------------------------------------------------------------
# Custom DVE op: fused broadcast-add + segmented max-scan.
#
# out[p, s, n] = running max over n' <= n of (in0[p, s, n'] + in1[p, s, n'])
# with the running max RESET at each page boundary, so out[p, s, N-1] =
# max_n (in0[p,s,n] + in1[p,s,n]) -- a fused add + segmented max reduce in
# one 1-elem/cycle DVE pass.
#
# The stock dve_spec.lower() only emits a SUB_DIM step state for
# PageIdx-style scans; a plain scan() accumulates globally across pages.
# Here we hand-build the FSM: steady fires SUB_DIM_DONE -> step; the step
# state (one element, the first of the new page) computes
# d = MAX(MaxNeg, expr) = expr, i.e. reset-and-include, then returns to
# steady.  Verified bit-exact on hardware (see session microbench).
# --------------------------------------------------------------------------


def _np_segmax(in0, in1, c0, c1, c2):
    s = in0 + in1
    return np.maximum.accumulate(s, axis=-1)


def _register_segmax():
    from concourse import dve_spec as Dv
    from concourse import dve_ops as DO
    from concourse.dve_spec import Spec, Src0, Src1, scan, AluOp
    from concourse.dve_uop import DveOpSpec

    for op in DO.OPS:
        if op.name == "SEG_MAXSCAN_ANT":
            return op

    def _lower_segmented(spec, ver):
        Dv._validate_body(spec, ver)
        spec2 = Dv._hoist_stream_invariant_ops(spec)
        scans = Dv._collect(spec2.body, Dv.Scan)
        latches = Dv._collect(spec2.body, Dv.Latch)
        assert not latches
        p = Dv._build_placement(spec2, scans, Dv.N_STAGES[ver], Dv.N_LANES[ver])
        states = list(Dv._build_state_machine(spec2, scans, latches, p))
        assert len(states) == 2, states  # [seed, steady]
        consume = states[1].consume

        step_ov = {}
        for sc in scans:
            d = p.node_stage[sc]
            init = Dv._scan_init(sc)
            # steady stage is _Stage(op, CURR_ALU_OUT, sc.expr); replace the
            # accumulator operand with the init leaf: d = op(init, expr)
            step_ov[d] = Dv._Stage(sc.op, init, sc.expr)

        steady_idx, step_idx = 1, 2
        states[steady_idx] = Dv._State(
            placement=p,
            consume=consume,
            trigger=(Dv.Trigger.SRC_TENSOR_DONE, Dv.Trigger.SUB_DIM_DONE,
                     Dv.Trigger.NONE),
            next=(0, step_idx, 0),
        )
        states.append(
            Dv._State(
                placement=p,
                consume=consume,
                overrides=step_ov,
                trigger=(Dv.Trigger.SRC_TENSOR_DONE, Dv.Trigger.SUB_DIM_DONE,
                         Dv.Trigger.COUNT),
                next=(0, step_idx, steady_idx),
                repeat=1,
            )
        )
        out = [Dv._assemble(s) for s in states]
        for u in out:
            u.validate(ver)
        return out

    class SegDveOp(DO.DveOp):
        def compile(self, ver):
            key = (self.name, ver)
            if (r := DO._COMPILE_CACHE.get(key)) is not None:
                return r
            result = DveOpSpec(
                name=self.name,
                opcode=DO.get_dve_sub_opcode(self.name),
                uops=_lower_segmented(self.spec, ver),
                rd1_en=DO.has_src1(self.spec),
            )
            DO._COMPILE_CACHE[key] = result
            return result

    spec = Spec(body=scan(AluOp.MAX, Src0 + Src1), reference=_np_segmax)
    op = SegDveOp("SEG_MAXSCAN_ANT", spec, subdim=True, uops_sha={})
    DO.OPS.append(op)
    DO._SUB_OPCODE_FOR_NAME[op.name] = DO._CUSTOM_DVE_ROW_BASE + len(DO.OPS) - 1
    assert DO._SUB_OPCODE_FOR_NAME[op.name] < 0x20
    return op


def _build(s_len):
    import concourse.bass as bass
    import concourse.bacc as bacc
    import concourse.tile as tile
    import concourse.mybir as mybir

    segmax = _register_segmax()

    alu = mybir.AluOpType
    f32 = mybir.dt.float32
    i32 = mybir.dt.int32

    nc = bacc.Bacc("TRN2", target_bir_lowering=False, debug=False)
    Xh = nc.dram_tensor("X", (BC, s_len, D), f32, kind="ExternalInput")
    Wh = nc.dram_tensor("W", (D, L), f32, kind="ExternalInput")
    Th = nc.dram_tensor("T", (L, L), f32, kind="ExternalInput")
    Oh = nc.dram_tensor("OUT", (BC, s_len, L), f32, kind="ExternalOutput")

    SCHUNK = 8 if s_len % 16 == 0 else s_len   # X staging granularity
    NCH = s_len // SCHUNK
    HALF = s_len // 2
    CH = 64 if s_len % 64 == 0 else s_len      # tail chunk (steps)
    NP = s_len - 1                              # number of fused pairs
    CS = s_len                                  # c-slot base index in d_store

    def ap_of(t, offset_elems, dims):
        a = t[:]
        return bass.AP(tensor=a.tensor, offset=a.offset + offset_elems,
                       ap=[list(a.ap[0])] + dims)

    with tile.TileContext(nc) as tc:
        with (
            tc.tile_pool(name="singles", bufs=1) as singles,
            tc.tile_pool(name="xstage", bufs=2) as xstage_p,
            tc.tile_pool(name="xt", bufs=3) as xt_p,
            tc.tile_pool(name="ps_t", bufs=2, space="PSUM") as ps_t,
            tc.tile_pool(name="ps_e", bufs=2, space="PSUM") as ps_e,
            tc.tile_pool(name="scores", bufs=3) as scores_p,
            tc.tile_pool(name="tail", bufs=2) as tail_p,
        ):
            # ---- storage ----
            e_store = singles.tile([BC, s_len * L], f32)   # emissions; then gammas
            d_store = singles.tile([BC, (s_len + 2) * L], f32)  # deltas + 2 c-slots
            b_store = singles.tile([BC, (s_len - HALF) * L], f32)  # beta, t >= HALF
            w_sb = singles.tile([D, L], f32)
            nc.sync.dma_start(w_sb[:], Wh[:])

            # T_cat[p, 0, j, i] = T[i, j] (fwd: pages i, elems j over T^T)
            # T_cat[p, 1, i, j] = T[i, j] (bwd: pages i, elems j over T)
            # NOTE: the per-column T^T gather below costs ~87K 4-byte DMA
            # descriptors (~100us of queue work) before the first X chunk
            # lands, delaying scan start.  Replacing it with an on-chip
            # strided copy starts the scan at ~17us BUT systematically
            # inflates every scan-loop Vector op ~8-20% (scheduler/semaphore
            # layout shift, net +135us).  Measured: gather-DMA 1.239ms vs
            # on-chip 1.374ms -- so the gather version is kept.
            t_ap = Th[:]
            t_cat = singles.tile([BC, 2, L, L], f32)
            nc.sync.dma_start(
                t_cat[:, 1, :, :].rearrange("p a b -> p (a b)"),
                bass.AP(tensor=t_ap.tensor, offset=t_ap.offset,
                        ap=[[0, BC], [1, L * L]]),
            )
            for j in range(L):
                nc.sync.dma_start(
                    t_cat[:, 0, j, :],
                    bass.AP(tensor=t_ap.tensor, offset=t_ap.offset + j,
                            ap=[[0, BC], [L, L]]),
                )

            # identity matrix for PE transpose: ident[p, q] = (p == q)
            idx_i = singles.tile([BC, D], i32)
            nc.gpsimd.iota(idx_i[:], pattern=[[1, D]], base=0, channel_multiplier=0)
            pid_i = singles.tile([BC, 1], i32)
            nc.gpsimd.iota(pid_i[:], pattern=[[0, 1]], base=0, channel_multiplier=1)
            idx_f = singles.tile([BC, D], f32)
            nc.vector.tensor_copy(idx_f[:], idx_i[:])
            pid_f = singles.tile([BC, 1], f32)
            nc.vector.tensor_copy(pid_f[:], pid_i[:])
            ident = singles.tile([BC, D], f32)
            nc.vector.tensor_scalar(
                out=ident[:], in0=idx_f[:], scalar1=pid_f[:], scalar2=None,
                op0=alu.is_equal,
            )

            e3 = e_store.rearrange("p (s l) -> p s l", l=L)
            d3 = d_store.rearrange("p (s l) -> p s l", l=L)
            b3 = b_store.rearrange("p (s l) -> p s l", l=L)

            # ---- Phase A: emissions, front/back interleaved chunk order ----
            order = []
            for c in range(NCH // 2):
                order += [c, NCH - 1 - c]
            if NCH % 2:
                order.append(NCH // 2)
            for cidx in order:
                c0 = cidx * SCHUNK
                xs = xstage_p.tile([BC, SCHUNK, D], f32)
                nc.sync.dma_start(xs[:], Xh[:, c0:c0 + SCHUNK, :])
                for si in range(SCHUNK):
                    s = c0 + si
                    xt_psum = ps_t.tile([D, BC], f32)
                    nc.tensor.transpose(xt_psum[:], xs[:, si, :], ident[:])
                    xt_sb = xt_p.tile([D, BC], f32)
                    nc.scalar.copy(xt_sb[:], xt_psum[:])
                    e_psum = ps_e.tile([BC, L], f32)
                    nc.tensor.matmul(e_psum[:], lhsT=xt_sb[:], rhs=w_sb[:],
                                     start=True, stop=True)
                    nc.scalar.copy(e3[:, s, :], e_psum[:])

            # ---- init: delta_0 = e_0; c-slot(0) = beta_{S-1} + e_{S-1} = e_{S-1}
            nc.vector.tensor_copy(d3[:, 0, :], e3[:, 0, :])
            nc.vector.tensor_copy(d3[:, CS, :], e3[:, s_len - 1, :])

            # ---- fused forward/backward scan pairs ----
            # DVE per pair: 2 custom segmax ops + 1 fused [2L] add.
            # GpSimd: beta persist (first half) + gamma = delta + beta parking
            # into dead e-slots (second half).
            for k in range(NP):
                ft = k + 1          # forward step being produced (delta_ft)
                bt = s_len - 2 - k  # backward step being produced (beta_bt)
                cin = CS + (k % 2)
                cout = CS + ((k + 1) % 2)

                sc = scores_p.tile([BC, 2, L, L], f32, tag="sc")
                in1_f = ap_of(d_store, k * L, [[0, L], [1, L]])
                in1_b = ap_of(d_store, cin * L, [[0, L], [1, L]])
                nc.vector._custom_dve(segmax, out=sc[:, 0], in0=t_cat[:, 0],
                                      in1=in1_f)
                nc.vector._custom_dve(segmax, out=sc[:, 1], in0=t_cat[:, 1],
                                      in1=in1_b)

                # [delta_ft | c_next] = page-end maxes + [e_ft | e_bt]
                ends = ap_of(sc, L - 1, [[L * L, 2], [L, L]])
                out_ap = ap_of(d_store, ft * L, [[(cout - ft) * L, 2], [1, L]])
                e_ap = ap_of(e_store, ft * L, [[(bt - ft) * L, 2], [1, L]])
                nc.vector.tensor_tensor(out_ap, ends, e_ap, op=alu.add)

                mxb = ap_of(sc, L * L + L - 1, [[L, L]])  # bwd page-end maxes
                if bt >= HALF:
                    # persist beta_bt for the forward side's gamma later
                    nc.gpsimd.tensor_copy(b3[:, bt - HALF, :], mxb)
                else:
                    # delta_bt is known: gamma_bt = delta_bt + beta_bt
                    nc.gpsimd.tensor_tensor(e3[:, bt, :], d3[:, bt, :], mxb,
                                            op=alu.add)
                if k >= HALF - 1 and ft <= s_len - 2:
                    # gamma_ft = delta_ft + beta_ft (beta from b_store)
                    nc.gpsimd.tensor_tensor(e3[:, ft, :], d3[:, ft, :],
                                            b3[:, ft - HALF, :], op=alu.add)

            # gamma_{S-1} = delta_{S-1} (beta = 0)
            nc.gpsimd.tensor_copy(e3[:, s_len - 1, :], d3[:, s_len - 1, :])

            # ---- Tail: onehot = (gamma >= rowmax(gamma)); DMA out ----
            # Vector does the segmented rowmax (only it can); the IS_GE of
            # early chunks goes to GpSimd so the two engines overlap.
            nchunks = s_len // CH
            for ci, c0 in enumerate(range(0, s_len, CH)):
                gsrc = e3[:, c0:c0 + CH, :]
                gm = tail_p.tile([BC, CH], f32, tag="gm")
                nc.vector.reduce_max(gm[:], gsrc, axis=mybir.AxisListType.X)
                oh = tail_p.tile([BC, CH, L], f32, tag="oh")
                gm_bc = (
                    gm[:]
                    .rearrange("p (t o) -> p t o", o=1)
                    .broadcast_to((BC, CH, L))
                )
                nc.vector.tensor_tensor(oh[:], gsrc, gm_bc, op=alu.is_ge)
                nc.sync.dma_start(Oh[:, c0:c0 + CH, :], oh[:])

    nc.compile()
    return nc


def _get(s_len):
    if s_len not in _BUILD_CACHE:
        _BUILD_CACHE[s_len] = _build(s_len)
    return _BUILD_CACHE[s_len]


LAST_RESULT = None


def kernel(X, W, T):
    global LAST_RESULT
    from concourse.bass_utils import run_bass_kernel_spmd

    X = np.ascontiguousarray(X, dtype=np.float32)
    W = np.ascontiguousarray(W, dtype=np.float32)
    T = np.ascontiguousarray(T, dtype=np.float32)
    s_len = X.shape[1]
    nc = _get(s_len)
    in_maps = [
        {"X": X[c * BC:(c + 1) * BC], "W": W, "T": T} for c in range(NCORES)
    ]
    res = run_bass_kernel_spmd(nc, in_maps, core_ids=list(range(NCORES)))
    LAST_RESULT = res
    return np.concatenate([r["OUT"] for r in res.results], axis=0)


# revision 8
# speedup vs baseline: 1.1122x; 1.0012x over previous
"""CRF Viterbi decode kernel for Trainium2 (8 NeuronCores, data-parallel over batch).

Per core (128 sequences, batch on partitions):
  Phase A: DMA X slabs (front/back interleaved) -> PE transpose -> PE matmul with W
           -> emissions e[b, s, l] (ACT copies PSUM->SBUF).
  Fused scans: forward delta-scan and backward beta-scan advance in lockstep
           pairs.  Each direction's "broadcast-add + segmented max" is ONE
           custom DVE instruction (SEG_MAXSCAN_ANT: scan(MAX, Src0+Src1) with a
           hand-built SUB_DIM_DONE page-reset state), so a pair costs
           2x ~900ns + one [128,52] add instead of ~3.3us of stock ops.
           GpSimd persists beta (first half) and computes gamma = delta + beta
           in-place into dead e-slots (both halves) during the scan.
  Tail:    onehot = (gamma >= rowmax(gamma)); DMA out.

Matches the jax reference up to f32 summation-order fuzz in emissions.
"""

import numpy as np

B, S, D, L = 1024, 512, 128, 26
NCORES = 8
BC = B // NCORES  # 128 sequences per core

_BUILD_CACHE = {}


# --------------------------------------------------------------------------
# Custom DVE op: fused broadcast-add + segmented max-scan.
#
# out[p, s, n] = running max over n' <= n of (in0[p, s, n'] + in1[p, s, n'])
# with the running max RESET at each page boundary, so out[p, s, N-1] =
# max_n (in0[p,s,n] + in1[p,s,n]) -- a fused add + segmented max reduce in
# one 1-elem/cycle DVE pass.
#
# The stock dve_spec.lower() only emits a SUB_DIM step state for
# PageIdx-style scans; a plain scan() accumulates globally across pages.
# Here we hand-build the FSM: steady fires SUB_DIM_DONE -> step; the step
# state (one element, the first of the new page) computes
# d = MAX(MaxNeg, expr) = expr, i.e. reset-and-include, then returns to
# steady.  Verified bit-exact on hardware (see session microbench).
# --------------------------------------------------------------------------


def _np_segmax(in0, in1, c0, c1, c2):
    s = in0 + in1
    return np.maximum.accumulate(s, axis=-1)


def _register_segmax():
    from concourse import dve_spec as Dv
    from concourse import dve_ops as DO
    from concourse.dve_spec import Spec, Src0, Src1, scan, AluOp
    from concourse.dve_uop import DveOpSpec

    for op in DO.OPS:
        if op.name == "SEG_MAXSCAN_ANT":
            return op

    def _lower_segmented(spec, ver):
        Dv._validate_body(spec, ver)
        spec2 = Dv._hoist_stream_invariant_ops(spec)
        scans = Dv._collect(spec2.body, Dv.Scan)
        latches = Dv._collect(spec2.body, Dv.Latch)
        assert not latches
        p = Dv._build_placement(spec2, scans, Dv.N_STAGES[ver], Dv.N_LANES[ver])
        states = list(Dv._build_state_machine(spec2, scans, latches, p))
        assert len(states) == 2, states  # [seed, steady]
        consume = states[1].consume

        step_ov = {}
        for sc in scans:
            d = p.node_stage[sc]
            init = Dv._scan_init(sc)
            # steady stage is _Stage(op, CURR_ALU_OUT, sc.expr); replace the
            # accumulator operand with the init leaf: d = op(init, expr)
            step_ov[d] = Dv._Stage(sc.op, init, sc.expr)

        steady_idx, step_idx = 1, 2
        states[steady_idx] = Dv._State(
            placement=p,
            consume=consume,
            trigger=(Dv.Trigger.SRC_TENSOR_DONE, Dv.Trigger.SUB_DIM_DONE,
                     Dv.Trigger.NONE),
            next=(0, step_idx, 0),
        )
        states.append(
            Dv._State(
                placement=p,
                consume=consume,
                overrides=step_ov,
                trigger=(Dv.Trigger.SRC_TENSOR_DONE, Dv.Trigger.SUB_DIM_DONE,
                         Dv.Trigger.COUNT),
                next=(0, step_idx, steady_idx),
                repeat=1,
            )
        )
        out = [Dv._assemble(s) for s in states]
        for u in out:
            u.validate(ver)
        return out

    class SegDveOp(DO.DveOp):
        def compile(self, ver):
            key = (self.name, ver)
            if (r := DO._COMPILE_CACHE.get(key)) is not None:
                return r
            result = DveOpSpec(
                name=self.name,
                opcode=DO.get_dve_sub_opcode(self.name),
                uops=_lower_segmented(self.spec, ver),
                rd1_en=DO.has_src1(self.spec),
            )
            DO._COMPILE_CACHE[key] = result
            return result

    spec = Spec(body=scan(AluOp.MAX, Src0 + Src1), reference=_np_segmax)
    op = SegDveOp("SEG_MAXSCAN_ANT", spec, subdim=True, uops_sha={})
    DO.OPS.append(op)
    DO._SUB_OPCODE_FOR_NAME[op.name] = DO._CUSTOM_DVE_ROW_BASE + len(DO.OPS) - 1
    assert DO._SUB_OPCODE_FOR_NAME[op.name] < 0x20
    return op


def _build(s_len):
    import concourse.bass as bass
    import concourse.bacc as bacc
    import concourse.tile as tile
    import concourse.mybir as mybir

    segmax = _register_segmax()

    alu = mybir.AluOpType
    f32 = mybir.dt.float32
    i32 = mybir.dt.int32

    nc = bacc.Bacc("TRN2", target_bir_lowering=False, debug=False)
    Xh = nc.dram_tensor("X", (BC, s_len, D), f32, kind="ExternalInput")
    Wh = nc.dram_tensor("W", (D, L), f32, kind="ExternalInput")
    Th = nc.dram_tensor("T", (L, L), f32, kind="ExternalInput")
    Oh = nc.dram_tensor("OUT", (BC, s_len, L), f32, kind="ExternalOutput")

    SCHUNK = 8 if s_len % 16 == 0 else s_len   # X staging granularity
    NCH = s_len // SCHUNK
    HALF = s_len // 2
    CH = 64 if s_len % 64 == 0 else s_len      # tail chunk (steps)
    NP = s_len - 1                              # number of fused pairs
    CS = s_len                                  # c-slot base index in d_store

    def ap_of(t, offset_elems, dims):
        a = t[:]
        return bass.AP(tensor=a.tensor, offset=a.offset + offset_elems,
                       ap=[list(a.ap[0])] + dims)

    with tile.TileContext(nc) as tc:
        with (
            tc.tile_pool(name="singles", bufs=1) as singles,
            tc.tile_pool(name="xstage", bufs=2) as xstage_p,
            tc.tile_pool(name="xt", bufs=3) as xt_p,
            tc.tile_pool(name="ps_t", bufs=2, space="PSUM") as ps_t,
            tc.tile_pool(name="ps_e", bufs=2, space="PSUM") as ps_e,
            tc.tile_pool(name="scores", bufs=3) as scores_p,
            tc.tile_pool(name="tail", bufs=2) as tail_p,
        ):
            # ---- storage ----
            e_store = singles.tile([BC, s_len * L], f32)   # emissions; then gammas
            d_store = singles.tile([BC, (s_len + 2) * L], f32)  # deltas + 2 c-slots
            b_store = singles.tile([BC, (s_len - HALF) * L], f32)  # beta, t >= HALF
            w_sb = singles.tile([D, L], f32)
            nc.sync.dma_start(w_sb[:], Wh[:])

            # T_cat[p, 0, j, i] = T[i, j] (fwd: pages i, elems j over T^T)
            # T_cat[p, 1, i, j] = T[i, j] (bwd: pages i, elems j over T)
            # NOTE: the per-column T^T gather below costs ~87K 4-byte DMA
            # descriptors (~100us of queue work) before the first X chunk
            # lands, delaying scan start.  Replacing it with an on-chip
            # strided copy starts the scan at ~17us BUT systematically
            # inflates every scan-loop Vector op ~8-20% (scheduler/semaphore
            # layout shift, net +135us).  Measured: gather-DMA 1.239ms vs
            # on-chip 1.374ms -- so the gather version is kept.
            t_ap = Th[:]
            t_cat = singles.tile([BC, 2, L, L], f32)
            nc.sync.dma_start(
                t_cat[:, 1, :, :].rearrange("p a b -> p (a b)"),
                bass.AP(tensor=t_ap.tensor, offset=t_ap.offset,
                        ap=[[0, BC], [1, L * L]]),
            )
            for j in range(L):
                nc.sync.dma_start(
                    t_cat[:, 0, j, :],
                    bass.AP(tensor=t_ap.tensor, offset=t_ap.offset + j,
                            ap=[[0, BC], [L, L]]),
                )

            # identity matrix for PE transpose: ident[p, q] = (p == q)
            idx_i = singles.tile([BC, D], i32)
            nc.gpsimd.iota(idx_i[:], pattern=[[1, D]], base=0, channel_multiplier=0)
            pid_i = singles.tile([BC, 1], i32)
            nc.gpsimd.iota(pid_i[:], pattern=[[0, 1]], base=0, channel_multiplier=1)
            idx_f = singles.tile([BC, D], f32)
            nc.vector.tensor_copy(idx_f[:], idx_i[:])
            pid_f = singles.tile([BC, 1], f32)
            nc.vector.tensor_copy(pid_f[:], pid_i[:])
            ident = singles.tile([BC, D], f32)
            nc.vector.tensor_scalar(
                out=ident[:], in0=idx_f[:], scalar1=pid_f[:], scalar2=None,
                op0=alu.is_equal,
            )

            e3 = e_store.rearrange("p (s l) -> p s l", l=L)
            d3 = d_store.rearrange("p (s l) -> p s l", l=L)
            b3 = b_store.rearrange("p (s l) -> p s l", l=L)

            # ---- Phase A: emissions, front/back interleaved chunk order ----
            order = []
            for c in range(NCH // 2):
                order += [c, NCH - 1 - c]
            if NCH % 2:
                order.append(NCH // 2)
            for cidx in order:
                c0 = cidx * SCHUNK
                xs = xstage_p.tile([BC, SCHUNK, D], f32)
                nc.sync.dma_start(xs[:], Xh[:, c0:c0 + SCHUNK, :])
                for si in range(SCHUNK):
                    s = c0 + si
                    xt_psum = ps_t.tile([D, BC], f32)
                    nc.tensor.transpose(xt_psum[:], xs[:, si, :], ident[:])
                    xt_sb = xt_p.tile([D, BC], f32)
                    nc.scalar.copy(xt_sb[:], xt_psum[:])
                    e_psum = ps_e.tile([BC, L], f32)
                    nc.tensor.matmul(e_psum[:], lhsT=xt_sb[:], rhs=w_sb[:],
                                     start=True, stop=True)
                    nc.scalar.copy(e3[:, s, :], e_psum[:])

            # ---- init: delta_0 = e_0; c-slot(0) = beta_{S-1} + e_{S-1} = e_{S-1}
            nc.vector.tensor_copy(d3[:, 0, :], e3[:, 0, :])
            nc.vector.tensor_copy(d3[:, CS, :], e3[:, s_len - 1, :])

            # ---- tail emitter (used mid-scan and post-scan) ----
            # onehot chunk c: (gamma >= rowmax(gamma)); gammas live in e3.
            def emit_tail(c0):
                gsrc = e3[:, c0:c0 + CH, :]
                gm = tail_p.tile([BC, CH], f32, tag="gm")
                nc.vector.reduce_max(gm[:], gsrc, axis=mybir.AxisListType.X)
                oh = tail_p.tile([BC, CH, L], f32, tag="oh")
                gm_bc = (
                    gm[:]
                    .rearrange("p (t o) -> p t o", o=1)
                    .broadcast_to((BC, CH, L))
                )
                nc.vector.tensor_tensor(oh[:], gsrc, gm_bc, op=alu.is_ge)
                nc.sync.dma_start(Oh[:, c0:c0 + CH, :], oh[:])

            # chunk c's gammas are complete by scan step:
            #   c<4 (bwd side): k = NP-1 - 64c ;  c>=4 (fwd side): k = 64c+62
            tail_at = {}
            if s_len == 512:
                tail_at = {320: [3 * CH, 4 * CH], 384: [2 * CH, 5 * CH],
                           448: [1 * CH, 6 * CH]}

            # ---- fused forward/backward scan pairs ----
            # DVE per pair: 2 custom segmax ops + 1 fused [2L] add.
            # GpSimd: beta persist (first half) + gamma = delta + beta parking
            # into dead e-slots (second half).
            for k in range(NP):
                ft = k + 1          # forward step being produced (delta_ft)
                bt = s_len - 2 - k  # backward step being produced (beta_bt)
                cin = CS + (k % 2)
                cout = CS + ((k + 1) % 2)

                sc = scores_p.tile([BC, 2, L, L], f32, tag="sc")
                in1_f = ap_of(d_store, k * L, [[0, L], [1, L]])
                in1_b = ap_of(d_store, cin * L, [[0, L], [1, L]])
                nc.vector._custom_dve(segmax, out=sc[:, 0], in0=t_cat[:, 0],
                                      in1=in1_f)
                nc.vector._custom_dve(segmax, out=sc[:, 1], in0=t_cat[:, 1],
                                      in1=in1_b)

                # [delta_ft | c_next] = page-end maxes + [e_ft | e_bt]
                ends = ap_of(sc, L - 1, [[L * L, 2], [L, L]])
                out_ap = ap_of(d_store, ft * L, [[(cout - ft) * L, 2], [1, L]])
                e_ap = ap_of(e_store, ft * L, [[(bt - ft) * L, 2], [1, L]])
                nc.vector.tensor_tensor(out_ap, ends, e_ap, op=alu.add)

                mxb = ap_of(sc, L * L + L - 1, [[L, L]])  # bwd page-end maxes
                if bt >= HALF:
                    # persist beta_bt for the forward side's gamma later
                    nc.gpsimd.tensor_copy(b3[:, bt - HALF, :], mxb)
                else:
                    # delta_bt is known: gamma_bt = delta_bt + beta_bt
                    nc.gpsimd.tensor_tensor(e3[:, bt, :], d3[:, bt, :], mxb,
                                            op=alu.add)
                if k >= HALF - 1 and ft <= s_len - 2:
                    # gamma_ft = delta_ft + beta_ft (beta from b_store)
                    nc.gpsimd.tensor_tensor(e3[:, ft, :], d3[:, ft, :],
                                            b3[:, ft - HALF, :], op=alu.add)
                for c0 in tail_at.get(k, ()):
                    emit_tail(c0)

            # gamma_{S-1} = delta_{S-1} (beta = 0)
            nc.gpsimd.tensor_copy(e3[:, s_len - 1, :], d3[:, s_len - 1, :])

            # ---- Tail: remaining onehot chunks ----
            done = {c for cs in tail_at.values() for c in cs}
            for c0 in range(0, s_len, CH):
                if c0 not in done:
                    emit_tail(c0)

    nc.compile()
    return nc


def _get(s_len):
    if s_len not in _BUILD_CACHE:
        _BUILD_CACHE[s_len] = _build(s_len)
    return _BUILD_CACHE[s_len]


LAST_RESULT = None


def kernel(X, W, T):
    global LAST_RESULT
    from concourse.bass_utils import run_bass_kernel_spmd

    X = np.ascontiguousarray(X, dtype=np.float32)
    W = np.ascontiguousarray(W, dtype=np.float32)
    T = np.ascontiguousarray(T, dtype=np.float32)
    s_len = X.shape[1]
    nc = _get(s_len)
    in_maps = [
        {"X": X[c * BC:(c + 1) * BC], "W": W, "T": T} for c in range(NCORES)
    ]
    res = run_bass_kernel_spmd(nc, in_maps, core_ids=list(range(NCORES)))
    LAST_RESULT = res
    return np.concatenate([r["OUT"] for r in res.results], axis=0)


# revision 9
# speedup vs baseline: 1.1966x; 1.0758x over previous
"""CRF Viterbi decode kernel for Trainium2 (8 NeuronCores, data-parallel over batch).

Per core (128 sequences, batch on partitions):
  Phase A: DMA X slabs (front/back interleaved) -> PE transpose -> PE matmul with W
           -> emissions e[b, s, l] (ACT copies PSUM->SBUF).
  Fused scans: forward delta-scan and backward beta-scan advance in lockstep
           pairs.  Each direction's "broadcast-add + segmented max" is ONE
           custom DVE instruction (SEG_MAXSCAN_ANT: scan(MAX, Src0+Src1) with a
           hand-built SUB_DIM_DONE page-reset state), so a pair costs
           2x ~900ns + one [128,52] add instead of ~3.3us of stock ops.
           GpSimd persists beta (first half) and computes gamma = delta + beta
           in-place into dead e-slots (both halves) during the scan.
  Tail:    onehot = (gamma >= rowmax(gamma)); DMA out.

Matches the jax reference up to f32 summation-order fuzz in emissions.
"""

import numpy as np

B, S, D, L = 1024, 512, 128, 26
NCORES = 8
BC = B // NCORES  # 128 sequences per core

_BUILD_CACHE = {}


# --------------------------------------------------------------------------
# Custom DVE op: fused broadcast-add + segmented max-scan.
#
# out[p, s, n] = running max over n' <= n of (in0[p, s, n'] + in1[p, s, n'])
# with the running max RESET at each page boundary, so out[p, s, N-1] =
# max_n (in0[p,s,n] + in1[p,s,n]) -- a fused add + segmented max reduce in
# one 1-elem/cycle DVE pass.
#
# The stock dve_spec.lower() only emits a SUB_DIM step state for
# PageIdx-style scans; a plain scan() accumulates globally across pages.
# Here we hand-build the FSM: steady fires SUB_DIM_DONE -> step; the step
# state (one element, the first of the new page) computes
# d = MAX(MaxNeg, expr) = expr, i.e. reset-and-include, then returns to
# steady.  Verified bit-exact on hardware (see session microbench).
# --------------------------------------------------------------------------


def _np_segmax(in0, in1, c0, c1, c2):
    s = in0 + in1
    return np.maximum.accumulate(s, axis=-1)


def _register_segmax():
    from concourse import dve_spec as Dv
    from concourse import dve_ops as DO
    from concourse.dve_spec import Spec, Src0, Src1, scan, AluOp
    from concourse.dve_uop import DveOpSpec

    for op in DO.OPS:
        if op.name == "SEG_MAXSCAN_ANT":
            return op

    def _lower_segmented(spec, ver):
        Dv._validate_body(spec, ver)
        spec2 = Dv._hoist_stream_invariant_ops(spec)
        scans = Dv._collect(spec2.body, Dv.Scan)
        latches = Dv._collect(spec2.body, Dv.Latch)
        assert not latches
        p = Dv._build_placement(spec2, scans, Dv.N_STAGES[ver], Dv.N_LANES[ver])
        states = list(Dv._build_state_machine(spec2, scans, latches, p))
        assert len(states) == 2, states  # [seed, steady]
        consume = states[1].consume

        step_ov = {}
        for sc in scans:
            d = p.node_stage[sc]
            init = Dv._scan_init(sc)
            # steady stage is _Stage(op, CURR_ALU_OUT, sc.expr); replace the
            # accumulator operand with the init leaf: d = op(init, expr)
            step_ov[d] = Dv._Stage(sc.op, init, sc.expr)

        steady_idx, step_idx = 1, 2
        states[steady_idx] = Dv._State(
            placement=p,
            consume=consume,
            trigger=(Dv.Trigger.SRC_TENSOR_DONE, Dv.Trigger.SUB_DIM_DONE,
                     Dv.Trigger.NONE),
            next=(0, step_idx, 0),
        )
        states.append(
            Dv._State(
                placement=p,
                consume=consume,
                overrides=step_ov,
                trigger=(Dv.Trigger.SRC_TENSOR_DONE, Dv.Trigger.SUB_DIM_DONE,
                         Dv.Trigger.COUNT),
                next=(0, step_idx, steady_idx),
                repeat=1,
            )
        )
        out = [Dv._assemble(s) for s in states]
        for u in out:
            u.validate(ver)
        return out

    class SegDveOp(DO.DveOp):
        def compile(self, ver):
            key = (self.name, ver)
            if (r := DO._COMPILE_CACHE.get(key)) is not None:
                return r
            result = DveOpSpec(
                name=self.name,
                opcode=DO.get_dve_sub_opcode(self.name),
                uops=_lower_segmented(self.spec, ver),
                rd1_en=DO.has_src1(self.spec),
            )
            DO._COMPILE_CACHE[key] = result
            return result

    spec = Spec(body=scan(AluOp.MAX, Src0 + Src1), reference=_np_segmax)
    op = SegDveOp("SEG_MAXSCAN_ANT", spec, subdim=True, uops_sha={})
    DO.OPS.append(op)
    DO._SUB_OPCODE_FOR_NAME[op.name] = DO._CUSTOM_DVE_ROW_BASE + len(DO.OPS) - 1
    assert DO._SUB_OPCODE_FOR_NAME[op.name] < 0x20
    return op


def _build(s_len):
    import concourse.bass as bass
    import concourse.bacc as bacc
    import concourse.tile as tile
    import concourse.mybir as mybir

    segmax = _register_segmax()

    alu = mybir.AluOpType
    f32 = mybir.dt.float32
    i32 = mybir.dt.int32

    nc = bacc.Bacc("TRN2", target_bir_lowering=False, debug=False)
    Xh = nc.dram_tensor("X", (BC, s_len, D), f32, kind="ExternalInput")
    Wh = nc.dram_tensor("W", (D, L), f32, kind="ExternalInput")
    Th = nc.dram_tensor("T", (L, L), f32, kind="ExternalInput")
    Oh = nc.dram_tensor("OUT", (BC, s_len, L), f32, kind="ExternalOutput")

    SCHUNK = 8 if s_len % 16 == 0 else s_len   # X staging granularity
    NCH = s_len // SCHUNK
    HALF = s_len // 2
    CH = 64 if s_len % 64 == 0 else s_len      # tail chunk (steps)
    NP = s_len - 1                              # number of fused pairs
    CS = s_len                                  # c-slot base index in d_store

    def ap_of(t, offset_elems, dims):
        a = t[:]
        return bass.AP(tensor=a.tensor, offset=a.offset + offset_elems,
                       ap=[list(a.ap[0])] + dims)

    with tile.TileContext(nc) as tc:
        with (
            tc.tile_pool(name="singles", bufs=1) as singles,
            tc.tile_pool(name="xstage", bufs=2) as xstage_p,
            tc.tile_pool(name="xt", bufs=3) as xt_p,
            tc.tile_pool(name="ps_t", bufs=2, space="PSUM") as ps_t,
            tc.tile_pool(name="ps_e", bufs=2, space="PSUM") as ps_e,
            tc.tile_pool(name="scores", bufs=3) as scores_p,
            tc.tile_pool(name="tail", bufs=2) as tail_p,
        ):
            # ---- storage ----
            e_store = singles.tile([BC, s_len * L], f32)   # emissions; then gammas
            d_store = singles.tile([BC, (s_len + 2) * L], f32)  # deltas + 2 c-slots
            b_store = singles.tile([BC, (s_len - HALF) * L], f32)  # beta, t >= HALF
            w_sb = singles.tile([D, L], f32)
            nc.sync.dma_start(w_sb[:], Wh[:])

            # T_cat[p, 0, j, i] = T[i, j] (fwd: pages i, elems j over T^T)
            # T_cat[p, 1, i, j] = T[i, j] (bwd: pages i, elems j over T)
            # NOTE: the per-column T^T gather below costs ~87K 4-byte DMA
            # descriptors (~100us of queue work) before the first X chunk
            # lands, delaying scan start.  Replacing it with an on-chip
            # strided copy starts the scan at ~17us BUT systematically
            # inflates every scan-loop Vector op ~8-20% (scheduler/semaphore
            # layout shift, net +135us).  Measured: gather-DMA 1.239ms vs
            # on-chip 1.374ms -- so the gather version is kept.
            t_ap = Th[:]
            t_cat = singles.tile([BC, 2, L, L], f32)
            nc.sync.dma_start(
                t_cat[:, 1, :, :].rearrange("p a b -> p (a b)"),
                bass.AP(tensor=t_ap.tensor, offset=t_ap.offset,
                        ap=[[0, BC], [1, L * L]]),
            )
            # T^T: 4-byte transpose-gather into ONE partition (676
            # descriptors), then GpSimd daisy-chain broadcast to all 128.
            # (The old all-partition gather was 87K descriptors / ~100us of
            # DMA-queue jam that delayed the first X chunks and scan start.)
            t_flat = singles.tile([1, L * L], f32)
            nc.sync.dma_start(
                t_flat[:],
                bass.AP(tensor=t_ap.tensor, offset=t_ap.offset,
                        ap=[[0, 1], [1, L], [L, L]]),
            )
            nc.gpsimd.partition_broadcast(
                t_cat[:, 0, :, :].rearrange("p a b -> p (a b)"), t_flat[:])

            # identity matrix for PE transpose: ident[p, q] = (p == q)
            idx_i = singles.tile([BC, D], i32)
            nc.gpsimd.iota(idx_i[:], pattern=[[1, D]], base=0, channel_multiplier=0)
            pid_i = singles.tile([BC, 1], i32)
            nc.gpsimd.iota(pid_i[:], pattern=[[0, 1]], base=0, channel_multiplier=1)
            idx_f = singles.tile([BC, D], f32)
            nc.vector.tensor_copy(idx_f[:], idx_i[:])
            pid_f = singles.tile([BC, 1], f32)
            nc.vector.tensor_copy(pid_f[:], pid_i[:])
            ident = singles.tile([BC, D], f32)
            nc.vector.tensor_scalar(
                out=ident[:], in0=idx_f[:], scalar1=pid_f[:], scalar2=None,
                op0=alu.is_equal,
            )

            e3 = e_store.rearrange("p (s l) -> p s l", l=L)
            d3 = d_store.rearrange("p (s l) -> p s l", l=L)
            b3 = b_store.rearrange("p (s l) -> p s l", l=L)

            # ---- Phase A: emissions, front/back interleaved chunk order ----
            order = []
            for c in range(NCH // 2):
                order += [c, NCH - 1 - c]
            if NCH % 2:
                order.append(NCH // 2)
            for cidx in order:
                c0 = cidx * SCHUNK
                xs = xstage_p.tile([BC, SCHUNK, D], f32)
                nc.sync.dma_start(xs[:], Xh[:, c0:c0 + SCHUNK, :])
                for si in range(SCHUNK):
                    s = c0 + si
                    xt_psum = ps_t.tile([D, BC], f32)
                    nc.tensor.transpose(xt_psum[:], xs[:, si, :], ident[:])
                    xt_sb = xt_p.tile([D, BC], f32)
                    nc.scalar.copy(xt_sb[:], xt_psum[:])
                    e_psum = ps_e.tile([BC, L], f32)
                    nc.tensor.matmul(e_psum[:], lhsT=xt_sb[:], rhs=w_sb[:],
                                     start=True, stop=True)
                    nc.scalar.copy(e3[:, s, :], e_psum[:])

            # ---- init: delta_0 = e_0; c-slot(0) = beta_{S-1} + e_{S-1} = e_{S-1}
            nc.vector.tensor_copy(d3[:, 0, :], e3[:, 0, :])
            nc.vector.tensor_copy(d3[:, CS, :], e3[:, s_len - 1, :])

            # ---- tail emitter (used mid-scan and post-scan) ----
            # onehot chunk c: (gamma >= rowmax(gamma)); gammas live in e3.
            def emit_tail(c0):
                gsrc = e3[:, c0:c0 + CH, :]
                gm = tail_p.tile([BC, CH], f32, tag="gm")
                nc.vector.reduce_max(gm[:], gsrc, axis=mybir.AxisListType.X)
                oh = tail_p.tile([BC, CH, L], f32, tag="oh")
                gm_bc = (
                    gm[:]
                    .rearrange("p (t o) -> p t o", o=1)
                    .broadcast_to((BC, CH, L))
                )
                nc.vector.tensor_tensor(oh[:], gsrc, gm_bc, op=alu.is_ge)
                nc.sync.dma_start(Oh[:, c0:c0 + CH, :], oh[:])

            # chunk c's gammas are complete by scan step:
            #   c<4 (bwd side): k = NP-1 - 64c ;  c>=4 (fwd side): k = 64c+62
            tail_at = {}
            if s_len == 512:
                tail_at = {320: [3 * CH, 4 * CH], 384: [2 * CH, 5 * CH],
                           448: [1 * CH, 6 * CH]}

            # ---- fused forward/backward scan pairs ----
            # DVE per pair: 2 custom segmax ops + 1 fused [2L] add.
            # GpSimd: beta persist (first half) + gamma = delta + beta parking
            # into dead e-slots (second half).
            for k in range(NP):
                ft = k + 1          # forward step being produced (delta_ft)
                bt = s_len - 2 - k  # backward step being produced (beta_bt)
                cin = CS + (k % 2)
                cout = CS + ((k + 1) % 2)

                sc = scores_p.tile([BC, 2, L, L], f32, tag="sc")
                in1_f = ap_of(d_store, k * L, [[0, L], [1, L]])
                in1_b = ap_of(d_store, cin * L, [[0, L], [1, L]])
                nc.vector._custom_dve(segmax, out=sc[:, 0], in0=t_cat[:, 0],
                                      in1=in1_f)
                nc.vector._custom_dve(segmax, out=sc[:, 1], in0=t_cat[:, 1],
                                      in1=in1_b)

                # [delta_ft | c_next] = page-end maxes + [e_ft | e_bt]
                ends = ap_of(sc, L - 1, [[L * L, 2], [L, L]])
                out_ap = ap_of(d_store, ft * L, [[(cout - ft) * L, 2], [1, L]])
                e_ap = ap_of(e_store, ft * L, [[(bt - ft) * L, 2], [1, L]])
                nc.vector.tensor_tensor(out_ap, ends, e_ap, op=alu.add)

                mxb = ap_of(sc, L * L + L - 1, [[L, L]])  # bwd page-end maxes
                if bt >= HALF:
                    # persist beta_bt for the forward side's gamma later
                    nc.gpsimd.tensor_copy(b3[:, bt - HALF, :], mxb)
                else:
                    # delta_bt is known: gamma_bt = delta_bt + beta_bt
                    nc.gpsimd.tensor_tensor(e3[:, bt, :], d3[:, bt, :], mxb,
                                            op=alu.add)
                if k >= HALF - 1 and ft <= s_len - 2:
                    # gamma_ft = delta_ft + beta_ft (beta from b_store)
                    nc.gpsimd.tensor_tensor(e3[:, ft, :], d3[:, ft, :],
                                            b3[:, ft - HALF, :], op=alu.add)
                for c0 in tail_at.get(k, ()):
                    emit_tail(c0)

            # gamma_{S-1} = delta_{S-1} (beta = 0)
            nc.gpsimd.tensor_copy(e3[:, s_len - 1, :], d3[:, s_len - 1, :])

            # ---- Tail: remaining onehot chunks ----
            done = {c for cs in tail_at.values() for c in cs}
            for c0 in range(0, s_len, CH):
                if c0 not in done:
                    emit_tail(c0)

    nc.compile()
    return nc


def _get(s_len):
    if s_len not in _BUILD_CACHE:
        _BUILD_CACHE[s_len] = _build(s_len)
    return _BUILD_CACHE[s_len]


LAST_RESULT = None


def kernel(X, W, T):
    global LAST_RESULT
    from concourse.bass_utils import run_bass_kernel_spmd

    X = np.ascontiguousarray(X, dtype=np.float32)
    W = np.ascontiguousarray(W, dtype=np.float32)
    T = np.ascontiguousarray(T, dtype=np.float32)
    s_len = X.shape[1]
    nc = _get(s_len)
    in_maps = [
        {"X": X[c * BC:(c + 1) * BC], "W": W, "T": T} for c in range(NCORES)
    ]
    res = run_bass_kernel_spmd(nc, in_maps, core_ids=list(range(NCORES)))
    LAST_RESULT = res
    return np.concatenate([r["OUT"] for r in res.results], axis=0)
